# revision 43
# baseline (speedup 1.0000x reference)
"""DiT block Bass kernel for 8 TRN2 NeuronCores.

Core i -> (b = i//4, g = i%4): batch item b; head group 4g..4g+3; token
quarter [512g, 512g+512) of batch b.  Activations are hidden-major
("transposed", [hidden_chunk=128, tokens]) throughout; PE transposes at
entry (x) and exit (out).  Collectives: AllGather(4) for mod + h,
AllToAll(4) for ctx.  Matmuls bf16 with f32 PSUM accumulate; softmax is
computed without max-subtraction (scores are provably small) with the
relative bias applied multiplicatively post-exp from a host-precomputed
diagonal-shifted exp(bias) table.
"""
import contextlib
import time
import numpy as np
import ml_dtypes
import jax
from jax.sharding import Mesh, PartitionSpec
from jax.experimental.shard_map import shard_map

import concourse.bass as bass
import concourse.mybir as mybir
import concourse.tile as tile
from concourse import bacc
from concourse.bass2jax import _bass_exec_p, install_neuronx_cc_hook, partition_id_tensor

F32 = mybir.dt.float32
BF16 = mybir.dt.bfloat16
AF = mybir.ActivationFunctionType
OP = mybir.AluOpType
ts = bass.ts

B, N, HID = 2, 2048, 1024
NH, HD = 16, 64
MLPH = 4 * HID
NB, MAXD = 32, 128
P = 128
TT = 512
KC = HID // P          # 8
NBLK = N // P          # 16
EB_A = 1920
EB_J = 3968
EB_LO = 1408           # kept band of the full eb table (non-saturated diagonals)
EB_J2 = 1152           # band width: delta=blk-4*tau in [-1,4] -> col0=512-128*delta
RG4 = [[0, 1, 2, 3], [4, 5, 6, 7]]


# ---------------------------------------------------------------- host prep
def rel_bucket_np(d):
    nb = NB // 2
    buckets = np.where(d > 0, nb, 0).astype(np.int64)
    rp = np.abs(d)
    max_exact = nb // 2
    is_small = rp < max_exact
    log_ratio = np.log(np.maximum(rp, 1).astype(np.float32) / np.float32(max_exact))
    rpl = max_exact + (
        log_ratio / np.float32(np.log(MAXD / max_exact)) * (nb - max_exact)
    ).astype(np.int32)
    rpl = np.minimum(rpl, nb - 1)
    return buckets + np.where(is_small, rp, rpl)


def make_eb_tables(rel_table):
    d = np.arange(-(N - 1), N)
    buck = rel_bucket_np(d)
    p = np.arange(P)[:, None]
    j = np.arange(EB_J)[None, :]
    dd = p + EB_A - j
    valid = (dd >= -(N - 1)) & (dd <= N - 1)
    idx = np.clip(dd + (N - 1), 0, 2 * N - 2)
    ebs = np.zeros((NH, P, EB_J), dtype=np.float32)
    for h in range(NH):
        bvec = rel_table[buck, h].astype(np.float32)
        tab = np.exp(bvec)[idx]
        tab[~valid] = 1.0
        ebs[h] = tab
    return ebs.astype(ml_dtypes.bfloat16)


def make_in_maps(inputs):
    x = np.asarray(inputs["x"], np.float32)
    c = np.asarray(inputs["c"], np.float32)
    w_ada = np.asarray(inputs["w_ada"], np.float32)
    b_ada = np.asarray(inputs["b_ada"], np.float32)
    w_qkv = np.asarray(inputs["w_qkv"], np.float32)
    b_qkv = np.asarray(inputs["b_qkv"], np.float32)
    w_out = np.asarray(inputs["w_out"], np.float32)
    b_out = np.asarray(inputs["b_out"], np.float32)
    rel_table = np.asarray(inputs["rel_table"], np.float32)
    w_mlp1 = np.asarray(inputs["w_mlp1"], np.float32)
    b_mlp1 = np.asarray(inputs["b_mlp1"], np.float32)
    w_mlp2 = np.asarray(inputs["w_mlp2"], np.float32)
    b_mlp2 = np.asarray(inputs["b_mlp2"], np.float32)

    eb_all = make_eb_tables(rel_table)
    eb_band = eb_all[:, :, EB_LO:EB_LO + EB_J2]
    # saturated-diagonal bias constants: for delta=blk-4*tau outside [-1,4]
    # the bucket is constant over the whole 128x512 block (lo: d<=-385 ->
    # bucket 15, hi: d>=129 -> bucket 31); fold bias into exp's bias operand.
    bk = rel_bucket_np(np.arange(-(N - 1), N))
    # need constancy for d <= -129 (delta<=-2) and d >= 129 (delta>=5)
    assert np.all(bk[:N - 129] == bk[0]) and np.all(bk[N - 1 + 129:] == bk[-1])
    lnsat = np.zeros((P, 8), np.float32)
    ident = np.eye(P, dtype=np.float32)
    ones_col = np.ones((P, 1), np.float32)
    ones_row = np.ones((1, P), np.float32)
    bf = ml_dtypes.bfloat16

    maps = []
    for i in range(8):
        b, g = divmod(i, 4)
        qs, ks, vs = 256 * g, HID + 256 * g, 2 * HID + 256 * g
        w_qkv_s = np.concatenate(
            [w_qkv[:, qs:qs + 256], w_qkv[:, ks:ks + 256], w_qkv[:, vs:vs + 256]], 1)
        b_qk = np.concatenate([b_qkv[qs:qs + 256], b_qkv[ks:ks + 256]])
        bv = b_qkv[vs:vs + 256]
        for hl in range(4):
            lnsat[:, 2 * hl] = rel_table[int(bk[0]), 4 * g + hl]
            lnsat[:, 2 * hl + 1] = rel_table[int(bk[-1]), 4 * g + hl]
        maps.append({
            "x_own": np.ascontiguousarray(x[b, 512 * g:512 * (g + 1), :]),
            "c_own": np.ascontiguousarray(c[b][:, None]),
            "w_ada_s": np.ascontiguousarray(
                w_ada[:, 1536 * g:1536 * (g + 1)].reshape(KC, P, 1536)
                .astype(bf)),
            "b_ada_full": np.ascontiguousarray(
                b_ada.reshape(4, 12, P).transpose(2, 0, 1).reshape(P, 48)),
            "w_qk_r": np.ascontiguousarray(
                w_qkv_s[:, :512].reshape(KC, P, 4, P).transpose(2, 1, 0, 3)
                .astype(bf)),
            "w_v_r": np.ascontiguousarray(
                w_qkv_s[:, 512:].reshape(KC, P, 256).transpose(1, 0, 2)
                .astype(bf)),
            "b_qk_s": np.ascontiguousarray(b_qk.reshape(4, P).T),
            "b_v_bcast": np.ascontiguousarray(
                np.broadcast_to(bv[None, :], (P, 256)).astype(bf)),
            "w_out_s": np.ascontiguousarray(
                w_out[256 * g:256 * (g + 1), :].reshape(2, P, HID)
                .transpose(1, 0, 2).astype(bf)),
            "b_out_r": np.ascontiguousarray(b_out.reshape(KC, P).T),
            "w_mlp1": np.ascontiguousarray(
                w_mlp1.reshape(KC, P, MLPH // P, P).transpose(2, 1, 0, 3)
                .astype(bf)),
            "b_mlp1_r": np.ascontiguousarray(b_mlp1.reshape(MLPH // P, P).T),
            "w_mlp2": np.ascontiguousarray(
                w_mlp2.reshape(2, 16, P, KC, P).transpose(3, 0, 2, 1, 4)
                .astype(bf)),
            "b_mlp2_r": np.ascontiguousarray(b_mlp2.reshape(KC, P).T),
            "eb": np.ascontiguousarray(eb_band[4 * g:4 * g + 4]),
            "lnsat": np.ascontiguousarray(lnsat),
            "ident": ident,
            "ones_col": ones_col,
            "ones_row": ones_row,
        })
    return maps


def assemble_output(results):
    out = np.zeros((B, N, HID), np.float32)
    for i in range(8):
        b, g = divmod(i, 4)
        out[b, 512 * g:512 * (g + 1), :] = results[i]["out"]
    return out


# ---------------------------------------------------------------- builder
def build_kernel(sim=False):
    nc = bacc.Bacc("TRN2", target_bir_lowering=False, debug=False, num_devices=8)

    din = lambda nm, sh, dt=F32: nc.dram_tensor(nm, sh, dt, kind="ExternalInput")
    x_own = din("x_own", [TT, HID])
    c_own = din("c_own", [HID, 1])
    w_ada_s = din("w_ada_s", [KC, P, 1536], BF16)
    b_ada_full = din("b_ada_full", [P, 48])
    w_qk_r = din("w_qk_r", [4, P, KC, P], BF16)
    w_v_r = din("w_v_r", [P, KC, 256], BF16)
    b_qk_s = din("b_qk_s", [P, 4])
    b_v_bcast = din("b_v_bcast", [P, 256], BF16)
    w_out_s = din("w_out_s", [P, 2, HID], BF16)
    b_out_r = din("b_out_r", [P, KC])
    w_mlp1 = din("w_mlp1", [MLPH // P, P, KC, P], BF16)
    b_mlp1_r = din("b_mlp1_r", [P, MLPH // P])
    w_mlp2 = din("w_mlp2", [KC, 2, P, 16, P], BF16)
    b_mlp2_r = din("b_mlp2_r", [P, KC])
    eb_in = din("eb", [4, P, EB_J2], BF16)
    lnsat_in = din("lnsat", [P, 8])
    ident_in = din("ident", [P, P])
    ones_col_in = din("ones_col", [P, 1])
    ones_row_in = din("ones_row", [1, P])

    out_t = nc.dram_tensor("out", [TT, HID], F32, kind="ExternalOutput")

    with tile.TileContext(nc) as tc, contextlib.ExitStack() as ctx:
        const = ctx.enter_context(tc.tile_pool(name="const", bufs=1))
        pers = ctx.enter_context(tc.tile_pool(name="pers", bufs=1))
        big = ctx.enter_context(tc.tile_pool(name="big", bufs=1))
        work = ctx.enter_context(tc.tile_pool(name="work", bufs=3))
        wst = ctx.enter_context(tc.tile_pool(name="wst", bufs=4))
        dram = ctx.enter_context(tc.tile_pool(name="dram", bufs=1, space="DRAM"))
        ebp = ctx.enter_context(tc.tile_pool(name="ebp", bufs=2))
        ps_acc = ctx.enter_context(tc.tile_pool(name="ps_acc", bufs=4, space="PSUM"))
        ps_bc = ctx.enter_context(tc.tile_pool(name="ps_bc", bufs=2, space="PSUM"))
        ps_ctx = ctx.enter_context(tc.tile_pool(name="ps_ctx", bufs=2, space="PSUM"))

        # ---------------- constants (ident first on SP so x transposes start
        # ASAP; small consts dispatched from the idle Act/DVE queues)
        ident = const.tile([P, P], F32)
        nc.sync.dma_start(ident[:], ident_in.ap())
        ones_col = const.tile([P, 1], F32)
        nc.scalar.dma_start(ones_col[:], ones_col_in.ap())
        F32R = mybir.dt.float32r
        ones_row = const.tile([1, P], F32R)
        nc.scalar.dma_start(ones_row[:], ones_row_in.ap().bitcast(F32R))
        b_qk_sb = const.tile([P, 4], F32)
        nc.scalar.dma_start(b_qk_sb[:], b_qk_s.ap())
        b_v_sb = const.tile([P, 256], BF16)
        nc.scalar.dma_start(b_v_sb[:], b_v_bcast.ap())
        b_out_sb = const.tile([P, KC], F32)
        nc.scalar.dma_start(b_out_sb[:], b_out_r.ap())
        b_mlp1_sb = const.tile([P, MLPH // P], F32)
        nc.gpsimd.dma_start(b_mlp1_sb[:], b_mlp1_r.ap())
        b_mlp2_sb = const.tile([P, KC], F32)
        nc.gpsimd.dma_start(b_mlp2_sb[:], b_mlp2_r.ap())
        b_ada_sb = const.tile([P, 48], F32)
        nc.gpsimd.dma_start(b_ada_sb[:], b_ada_full.ap())
        lnsat_sb = const.tile([P, 8], F32)
        nc.gpsimd.dma_start(lnsat_sb[:], lnsat_in.ap())
        ones_col_b = const.tile([P, 1], BF16)
        nc.vector.memset(ones_col_b[:], 1.0)
        eps_sb = const.tile([1, 1], F32)
        nc.vector.memset(eps_sb[:], 1e-6)

        # ---------------- phase 1a: xT via PE transpose (issued first: the x
        # row DMAs head the SP queue so PE starts ~1.5us in)
        xT = pers.tile([P, KC, TT], F32)
        for r in range(TT // P):
            x_sb = work.tile([P, HID], F32, tag="xrow", bufs=3)
            nc.sync.dma_start(x_sb[:], x_own.ap()[ts(r, P), :])
            for kc in range(KC):
                tps = ps_acc.tile([P, P], F32, tag="acc")
                nc.tensor.transpose(tps[:], x_sb[:, ts(kc, P)], ident[:])
                nc.vector.tensor_copy(xT[:, kc, ts(r, P)], tps[:])

        # ---------------- phase 0: mod shard, silu(c) stationary so the PE
        # streams w_ada (no unpipelined 128-cycle weight loads)
        cT_sb = pers.tile([P, KC], F32)
        nc.sync.dma_start(cT_sb[:], c_own.ap().rearrange("(c p) o -> p (c o)", p=P))
        silu_sb = pers.tile([P, KC], BF16)
        nc.scalar.activation(silu_sb[:], cT_sb[:], AF.Silu)
        wada_sb = big.tile([P, KC, 1536], BF16, tag="slot32")
        nc.sync.dma_start(wada_sb[:], w_ada_s.ap().rearrange("c p j -> p c j"))
        mod_bounce_in = dram.tile([1, 1536], F32)
        for j in range(3):
            mps = ps_acc.tile([1, TT], F32, tag="acc")
            for kc in range(KC):
                nc.tensor.matmul(mps[:], silu_sb[:, kc:kc + 1],
                                 wada_sb[:, kc, ts(j, TT)],
                                 start=(kc == 0), stop=(kc == KC - 1))
            mrow = work.tile([1, TT], F32, tag="rowtmp", bufs=4)
            nc.vector.tensor_copy(mrow[:], mps[:])
            nc.sync.dma_start(mod_bounce_in[:][:, ts(j, TT)], mrow[:])
        mod_bounce_out = dram.tile([4, 1536], F32)
        if sim:
            nc.sync.dma_start(mod_bounce_out[:][0:1, :], mod_bounce_in[:])
        else:
            nc.gpsimd.collective_compute(
                "AllGather", OP.bypass, replica_groups=RG4,
                ins=[mod_bounce_in.opt()], outs=[mod_bounce_out.opt()])
        mod_raw = pers.tile([P, 48], F32)
        nc.sync.dma_start(
            mod_raw[:], mod_bounce_out[:].rearrange("g (c p) -> p (g c)", p=P))
        mod_sb = pers.tile([P, 48], F32)
        nc.vector.tensor_add(mod_sb[:], mod_raw[:], b_ada_sb[:])

        def mod_chunk(vec_idx, kc):
            gc = 8 * vec_idx + kc
            return mod_sb[:, gc:gc + 1]

        sc1p_msa = pers.tile([P, KC], F32)
        sc1p_mlp = pers.tile([P, KC], F32)
        for kc in range(KC):
            nc.vector.tensor_scalar_add(sc1p_msa[:, kc:kc + 1], mod_chunk(1, kc), 1.0)
            nc.vector.tensor_scalar_add(sc1p_mlp[:, kc:kc + 1], mod_chunk(4, kc), 1.0)

        def ln_stats(src, tag):
            sum_ps = ps_acc.tile([1, TT], F32, tag="acc")
            for kc in range(KC):
                nc.tensor.matmul(sum_ps[:], ones_col[:], src[:, kc, :],
                                 start=(kc == 0), stop=(kc == KC - 1))
            sumsq_ps = ps_acc.tile([1, TT], F32, tag="acc")
            for kc in range(KC):
                sq = work.tile([P, TT], BF16, tag="wbf", bufs=6)
                nc.scalar.activation(sq[:], src[:, kc, :], AF.Square)
                nc.tensor.matmul(sumsq_ps[:], ones_col_b[:], sq[:],
                                 start=(kc == 0), stop=(kc == KC - 1))
            m_row = work.tile([1, TT], F32R, tag="rowtmp", bufs=4)
            nc.vector.tensor_scalar_mul(m_row[:], sum_ps[:], 1.0 / HID)
            msq = work.tile([1, TT], F32, tag="rowtmp", bufs=4)
            nc.vector.tensor_tensor(
                msq[:], m_row[:].bitcast(F32), m_row[:].bitcast(F32), op=OP.mult)
            var_row = work.tile([1, TT], F32, tag="rowtmp", bufs=4)
            nc.vector.scalar_tensor_tensor(
                var_row[:], sumsq_ps[:], 1.0 / HID, msq[:],
                op0=OP.mult, op1=OP.subtract)
            sd_row = work.tile([1, TT], F32, tag="rowtmp", bufs=4)
            nc.scalar.activation(sd_row[:], var_row[:], AF.Sqrt, bias=eps_sb[:])
            r_row = work.tile([1, TT], F32R, tag="rowtmp", bufs=4)
            with nc.allow_low_precision("f32r row for 1cyc broadcast matmul"):
                nc.vector.reciprocal(r_row[:], sd_row[:])
            m_bc = ps_bc.tile([P, TT], F32, tag="bc")
            nc.tensor.matmul(m_bc[:], ones_row[:], m_row[:], start=True, stop=True)
            r_bc = ps_bc.tile([P, TT], F32, tag="bc")
            nc.tensor.matmul(r_bc[:], ones_row[:], r_row[:], start=True, stop=True)
            return m_bc, r_bc

        # ---------------- phase 2: hT own + AllGather
        m_bc, r_bc = ln_stats(xT, "ln1")
        hT_own = big.tile([P, KC, TT], BF16, tag="slot32")
        for kc in range(KC):
            t0 = work.tile([P, TT], F32, tag="wf32", bufs=5)
            nc.vector.tensor_sub(t0[:], xT[:, kc, :], m_bc[:])
            t1 = work.tile([P, TT], F32, tag="wf32", bufs=5)
            nc.vector.tensor_tensor(t1[:], t0[:], r_bc[:], op=OP.mult)
            nc.vector.tensor_scalar(
                hT_own[:, kc, :], t1[:], sc1p_msa[:, kc:kc + 1], mod_chunk(0, kc),
                op0=OP.mult, op1=OP.add)
        h_bounce_in_a = dram.tile([HID // 2, TT], BF16)
        h_bounce_in_b = dram.tile([HID // 2, TT], BF16)
        nc.sync.dma_start(
            h_bounce_in_a[:].rearrange("(c p) t -> p c t", p=P), hT_own[:, 0:4, :])
        nc.sync.dma_start(
            h_bounce_in_b[:].rearrange("(c p) t -> p c t", p=P), hT_own[:, 4:8, :])
        h_bounce_out_a = dram.tile([2 * HID, TT], BF16)
        h_bounce_out_b = dram.tile([2 * HID, TT], BF16)
        if sim:
            nc.sync.dma_start(h_bounce_out_a[:][0:HID // 2, :], h_bounce_in_a[:])
            nc.sync.dma_start(h_bounce_out_b[:][0:HID // 2, :], h_bounce_in_b[:])
        else:
            nc.gpsimd.collective_compute(
                "AllGather", OP.bypass, replica_groups=RG4,
                ins=[h_bounce_in_a.opt()], outs=[h_bounce_out_a.opt()])
            nc.gpsimd.collective_compute(
                "AllGather", OP.bypass, replica_groups=RG4,
                ins=[h_bounce_in_b.opt()], outs=[h_bounce_out_b.opt()])
        hT_full = big.tile([P, 32, TT], BF16, tag="slot32")
        for jq in range(4):
            nc.sync.dma_start(
                hT_full[:, KC * jq:KC * jq + 4, :],
                h_bounce_out_a[:][ts(jq, HID // 2), :].rearrange("(c p) t -> p c t", p=P))
            nc.sync.dma_start(
                hT_full[:, KC * jq + 4:KC * jq + 8, :],
                h_bounce_out_b[:][ts(jq, HID // 2), :].rearrange("(c p) t -> p c t", p=P))

        # prefetch both eb tables (used in phase 4) while qkv computes
        eb_sbs = []
        for a in range(2):
            t = ebp.tile([P, 2, EB_J2], BF16, tag="eb")
            nc.sync.dma_start(
                t[:], eb_in.ap()[2 * a:2 * a + 2].rearrange("h p j -> p h j"))
            eb_sbs.append(t)

        # ---------------- phase 3: qkv
        qT = pers.tile([P, 2, N], BF16)
        kT = pers.tile([P, 2, N], BF16)
        v_aug = pers.tile([P, NBLK, 260], BF16)
        nc.vector.memset(
            v_aug[:].rearrange("p b (h e) -> p b h e", h=4)[:, :, :, 64:65], 1.0)

        wvb = wst.tile([P, KC, 256], BF16, tag="wb")
        nc.sync.dma_start(wvb[:], w_v_r.ap())
        for blk in range(NBLK):
            ps = ps_acc.tile([P, 256], F32, tag="acc")
            for kc in range(KC):
                nc.tensor.matmul(
                    ps[:], hT_full[:, 8 * (blk // 4) + kc, ts(blk % 4, P)],
                    wvb[:, kc, :], start=(kc == 0), stop=(kc == KC - 1))
            vtmp = work.tile([P, 256], BF16, tag="wbf", bufs=6)
            nc.vector.tensor_copy(vtmp[:], ps[:])
            nc.vector.tensor_add(
                v_aug[:, blk, :].rearrange("p (h e) -> p h e", h=4)[:, :, 0:64],
                vtmp[:].rearrange("p (h e) -> p h e", h=4), b_v_sb[:].rearrange("p (h e) -> p h e", h=4))

        for mu in range(4):       # q chunks 0,1; k chunks 2,3
            wqb = wst.tile([P, KC, P], BF16, tag="wb")
            nc.sync.dma_start(wqb[:], w_qk_r.ap()[mu])
            for tau in range(4):
                ps = ps_acc.tile([P, TT], F32, tag="acc")
                for kc in range(KC):
                    nc.tensor.matmul(
                        ps[:], wqb[:, kc, :], hT_full[:, 8 * tau + kc, :],
                        start=(kc == 0), stop=(kc == KC - 1))
                dst = qT if mu < 2 else kT
                nc.vector.tensor_scalar_add(
                    dst[:, mu % 2, ts(tau, TT)], ps[:], b_qk_sb[:, mu:mu + 1])
        # ---------------- phase 4: attention
        ctxT = pers.tile([P, 2, N], BF16)
        for a in range(2):
            eb_sb = eb_sbs[a]
            for tau in range(4):
                cps0 = ps_ctx.tile([65, TT], F32, tag="ctx")
                cps1 = ps_ctx.tile([65, TT], F32, tag="ctx")
                cps = [cps0, cps1]
                for blk in range(NBLK):
                    delta = blk - 4 * tau
                    col0 = TT - P * delta
                    sat = delta < -1 or delta > 4
                    sps = []
                    for o in range(2):
                        sp = ps_acc.tile([P, TT], F32, tag="acc")
                        nc.tensor.matmul(
                            sp[:],
                            kT[64 * o:64 * o + 64, a, ts(blk, P)],
                            qT[64 * o:64 * o + 64, a, ts(tau, TT)],
                            start=True, stop=True)
                        sps.append(sp)
                    for o in range(2):
                        h = 2 * a + o
                        esb = work.tile([P, TT], BF16, tag="wbf", bufs=6)
                        if sat:
                            # constant bias over this block: fold into exp
                            sc = 2 * h + (1 if delta > 0 else 0)
                            nc.scalar.activation(
                                esb[:], sps[o][:], AF.Exp, scale=0.125,
                                bias=lnsat_sb[:, sc:sc + 1])
                        else:
                            tsb = work.tile([P, TT], BF16, tag="wbf", bufs=6)
                            nc.scalar.activation(
                                tsb[:], sps[o][:], AF.Exp, scale=0.125)
                            nc.vector.tensor_tensor(
                                esb[:], tsb[:], eb_sb[:, o, col0:col0 + TT],
                                op=OP.mult)
                        nc.tensor.matmul(
                            cps[o][:], v_aug[:, blk, 65 * h:65 * h + 65], esb[:],
                            start=(blk == 0), stop=(blk == NBLK - 1))
                for o in range(2):
                    recip = work.tile([1, TT], F32R, tag="rowtmp", bufs=4)
                    with nc.allow_low_precision("f32r row for 1cyc broadcast matmul"):
                        nc.vector.reciprocal(recip[:], cps[o][64:65, :])
                    bc = ps_bc.tile([64, TT], F32, tag="bc")
                    nc.tensor.matmul(bc[:], ones_row[:, 0:64], recip[:],
                                     start=True, stop=True)
                    csb = work.tile([64, TT], BF16, tag="wbf", bufs=6)
                    nc.vector.tensor_copy(csb[:], cps[o][0:64, :])
                    nc.vector.tensor_tensor(
                        ctxT[64 * o:64 * o + 64, a, ts(tau, TT)],
                        csb[:], bc[:], op=OP.mult)

        # ---------------- phase 5: head-sharded out-proj partials + RS(add)
        # partial attn_out^T over own 4 heads (ctx dims 256), ALL tokens
        wob = wst.tile([P, 2, HID], BF16, tag="wb")
        nc.sync.dma_start(wob[:], w_out_s.ap())
        po_sb = big.tile([P, KC, N], BF16, tag="slot32")
        for tau in range(4):
            for mu in range(KC):
                ps = ps_acc.tile([P, TT], F32, tag="acc")
                for kc in range(2):
                    nc.tensor.matmul(
                        ps[:], wob[:, kc, ts(mu, P)],
                        ctxT[:, kc, ts(tau, TT)],
                        start=(kc == 0), stop=(kc == 1))
                nc.vector.tensor_copy(po_sb[:, mu, ts(tau, TT)], ps[:])
        rs_bounce_in = dram.tile([4 * HID, TT], BF16)
        for j in range(4):
            nc.sync.dma_start(
                rs_bounce_in[:][ts(j, HID), :].rearrange("(c p) t -> p c t", p=P),
                po_sb[:, :, ts(j, TT)])
        rs_bounce_out = dram.tile([HID, TT], BF16)
        if sim:
            nc.sync.dma_start(rs_bounce_out[:], rs_bounce_in[:][0:HID, :])
        else:
            nc.gpsimd.collective_compute(
                "ReduceScatter", OP.add, replica_groups=RG4,
                ins=[rs_bounce_in.opt()], outs=[rs_bounce_out.opt()])
        ao_sb = pers.tile([P, KC, TT], BF16)
        nc.sync.dma_start(
            ao_sb[:], rs_bounce_out[:].rearrange("(c p) t -> p c t", p=P))

        # ---------------- phase 6: residual + LN2
        x2T = pers.tile([P, KC, TT], F32)
        for mu in range(KC):
            tmp = work.tile([P, TT], F32, tag="wf32", bufs=5)
            nc.vector.tensor_scalar(
                tmp[:], ao_sb[:, mu, :], b_out_sb[:, mu:mu + 1], mod_chunk(2, mu),
                op0=OP.add, op1=OP.mult)
            nc.vector.tensor_add(x2T[:, mu, :], tmp[:], xT[:, mu, :])

        m2_bc, r2_bc = ln_stats(x2T, "ln2")
        h2T = pers.tile([P, KC, TT], BF16)
        for kc in range(KC):
            t0 = work.tile([P, TT], F32, tag="wf32", bufs=5)
            nc.vector.tensor_sub(t0[:], x2T[:, kc, :], m2_bc[:])
            t1 = work.tile([P, TT], F32, tag="wf32", bufs=5)
            nc.vector.tensor_tensor(t1[:], t0[:], r2_bc[:], op=OP.mult)
            nc.vector.tensor_scalar(
                h2T[:, kc, :], t1[:], sc1p_mlp[:, kc:kc + 1], mod_chunk(3, kc),
                op0=OP.mult, op1=OP.add)

        # ---------------- phase 7: MLP (token-sharded, weights streamed)
        gT = big.tile([P, MLPH // P, TT], BF16, tag="slot32")
        for nu in range(MLPH // P):
            w1b = wst.tile([P, KC, P], BF16, tag="wb")
            nc.sync.dma_start(w1b[:], w_mlp1.ap()[nu])
            ps = ps_acc.tile([P, TT], F32, tag="acc")
            for kc in range(KC):
                nc.tensor.matmul(ps[:], w1b[:, kc, :], h2T[:, kc, :],
                                 start=(kc == 0), stop=(kc == KC - 1))
            nc.scalar.activation(
                gT[:, nu, :], ps[:], AF.Gelu_apprx_tanh, bias=b_mlp1_sb[:, nu:nu + 1])
        for mu in range(KC):
            ps = ps_acc.tile([P, TT], F32, tag="acc")
            for half in range(2):
                w2b = wst.tile([P, 16, P], BF16, tag="wb")
                nc.sync.dma_start(w2b[:], w_mlp2.ap()[mu, half])
                for kc in range(16):
                    gkc = 16 * half + kc
                    nc.tensor.matmul(ps[:], w2b[:, kc, :], gT[:, gkc, :],
                                     start=(gkc == 0), stop=(gkc == MLPH // P - 1))
            tmp = work.tile([P, TT], F32, tag="wf32", bufs=5)
            nc.vector.tensor_scalar(
                tmp[:], ps[:], b_mlp2_sb[:, mu:mu + 1], mod_chunk(5, mu),
                op0=OP.add, op1=OP.mult)
            outT = work.tile([P, TT], F32, tag="wf32", bufs=5)
            nc.vector.tensor_add(outT[:], tmp[:], x2T[:, mu, :])
            for r in range(TT // P):
                tps = ps_acc.tile([P, P], F32, tag="acc")
                nc.tensor.transpose(tps[:], outT[:, ts(r, P)], ident[:])
                osb = work.tile([P, P], F32, tag="osb", bufs=4)
                nc.vector.tensor_copy(osb[:], tps[:])
                nc.sync.dma_start(out_t.ap()[ts(r, P), ts(mu, P)], osb[:])

    nc.compile()
    return nc


# ---------------------------------------------------------------- runner



class SpmdRunner:
    def __init__(self, nc, n_cores):
        install_neuronx_cc_hook()
        self.nc = nc
        self.n_cores = n_cores
        partition_name = nc.partition_id_tensor.name if nc.partition_id_tensor else None
        in_names, out_names, out_avals = [], [], []
        for alloc in nc.m.functions[0].allocations:
            if not isinstance(alloc, mybir.MemoryLocationSet):
                continue
            name = alloc.memorylocations[0].name
            if alloc.kind == "ExternalInput":
                if name != partition_name:
                    in_names.append(name)
            elif alloc.kind == "ExternalOutput":
                out_names.append(name)
                out_avals.append(
                    jax.core.ShapedArray(tuple(alloc.tensor_shape), mybir.dt.np(alloc.dtype))
                )
        self.in_names, self.out_names, self.out_avals = in_names, out_names, out_avals
        n_params = len(in_names)
        n_outs = len(out_avals)
        all_in_names = list(in_names) + list(out_names)
        if partition_name is not None:
            all_in_names.append(partition_name)

        def _body(*args):
            operands = list(args)
            if partition_name is not None:
                operands.append(partition_id_tensor())
            return tuple(
                _bass_exec_p.bind(
                    *operands,
                    out_avals=tuple(out_avals),
                    in_names=tuple(all_in_names),
                    out_names=tuple(out_names),
                    lowering_input_output_aliases=(),
                    sim_require_finite=True,
                    sim_require_nnan=True,
                    nc=nc,
                )
            )

        devices = jax.devices()[:n_cores]
        self.mesh = Mesh(np.asarray(devices), ("core",))
        donate = tuple(range(n_params, n_params + n_outs))
        self.fn = jax.jit(
            shard_map(
                _body,
                mesh=self.mesh,
                in_specs=(PartitionSpec("core"),) * (n_params + n_outs),
                out_specs=(PartitionSpec("core"),) * n_outs,
                check_rep=False,
            ),
            donate_argnums=donate,
            keep_unused=True,
        )
        self.n_params, self.n_outs = n_params, n_outs

    def _concat_inputs(self, in_maps):
        return [
            np.concatenate([np.asarray(in_maps[c][n]) for c in range(self.n_cores)], axis=0)
            for n in self.in_names
        ]

    def run(self, in_maps):
        sharding = jax.sharding.NamedSharding(self.mesh, PartitionSpec("core"))
        concat_in = [
            jax.device_put(x, sharding) for x in self._concat_inputs(in_maps)
        ]
        zeros = [
            jax.device_put(
                np.zeros((self.n_cores * a.shape[0], *a.shape[1:]), a.dtype), sharding)
            for a in self.out_avals
        ]
        outs = self.fn(*concat_in, *zeros)
        return self._split(outs)

    def _split(self, out_arrs):
        return [
            {
                n: np.asarray(out_arrs[i]).reshape(self.n_cores, *self.out_avals[i].shape)[c]
                for i, n in enumerate(self.out_names)
            }
            for c in range(self.n_cores)
        ]

    def bench(self, in_maps, iters=30, warmup=3):
        """Chained repeated execution: output buffers of call i are donated as
        the output operands of call i+1, serializing calls on-device."""
        sharding = jax.sharding.NamedSharding(self.mesh, PartitionSpec("core"))
        concat_in = [jax.device_put(x, sharding) for x in self._concat_inputs(in_maps)]
        outs = tuple(
            jax.device_put(
                np.zeros((self.n_cores * a.shape[0], *a.shape[1:]), a.dtype), sharding)
            for a in self.out_avals
        )
        for _ in range(warmup):
            outs = self.fn(*concat_in, *outs)
        jax.block_until_ready(outs)
        t0 = time.perf_counter()
        for _ in range(iters):
            outs = self.fn(*concat_in, *outs)
        jax.block_until_ready(outs)
        t1 = time.perf_counter()
        return (t1 - t0) / iters, self._split(outs)


_CACHE = {}


def kernel(**inputs):
    """Full-input DiT block on 8 NeuronCores; returns full [B, N, HID] f32."""
    if "nc" not in _CACHE:
        _CACHE["nc"] = build_kernel()
        _CACHE["runner"] = SpmdRunner(_CACHE["nc"], 8)
    maps = make_in_maps(inputs)
    results = _CACHE["runner"].run(maps)
    return assemble_output(results)

